# revision 17
# baseline (speedup 1.0000x reference)
"""BinaryTreeLSTMCell fused kernel for 8 TRN2 NeuronCores.

Strategy: 2D sharding — 4-way data-parallel over the batch x 2-way
tensor-parallel over the hidden (output) dim, so each core computes a
(2048 batch, 512 h) block of the output with no cross-core traffic.
Per core, gates^T = W_half @ [x|h_left|h_right]^T runs in fp8-E4M3
with perf_mode=DoubleRow (2 weights per PE cell, K=256 per matmul,
~1.8x bf16 throughput). Weights are pre-scaled by 2^10 on host so
their U(-1/sqrt(3072), ..) range clears e4m3's min-normal (0.0156);
the 2^-10 dequant rides the activation's scale operand together with
the bias add. Loop order k2-outer / moving-block-inner amortizes each
256-column LDWEIGHTS over 4 moving blocks. Gate nonlinearities fused
on ScalarE out of PSUM, LSTM cell elementwise on VectorE in fp32.
"""

import numpy as np
import ml_dtypes

import concourse.bacc as bacc
import concourse.mybir as mybir
import concourse.tile as tile
from concourse.bass_utils import run_bass_kernel_spmd

F32 = mybir.dt.float32
F8 = mybir.dt.float8e4
E4NP = ml_dtypes.float8_e4m3
AF = mybir.ActivationFunctionType
DR = mybir.MatmulPerfMode.DoubleRow

N_CORES = 8
B = 8192
IN_SIZE = 1024
HID = 1024
COMB = IN_SIZE + 2 * HID          # 3072 contraction dim
DP = 4                            # batch shards
TP = 2                            # hidden shards
BS = B // DP                      # 2048 batch rows per core
HS = HID // TP                    # 512 h-cols per core
KT = COMB // 128                  # 24 k-tiles
K2T = KT // 2                     # 12 double-k-tiles (256 each)
NT = 5 * HS // 128                # 20 gate tiles per core
JT = HS // 128                    # 4 h-subtiles per core
BBT = BS // 512                   # 4 moving blocks of 512
SW = 2.0 ** -10                   # weight dequant scale

_NC = {}


def _build(repeat=1):
    if repeat in _NC:
        return _NC[repeat]

    nc = bacc.Bacc("TRN2", target_bir_lowering=False, debug=False)

    # comb8[k2] = [128 kpart, 2 kplanes, BS batch] fp8 per double-k-tile.
    comb8 = nc.dram_tensor("comb8", [K2T, 128, 2, BS], F8, kind="ExternalInput").ap()
    # wbig8[n] = [128 kpart, 12 k2, 2 kplane, 128 m] fp8 per gate tile.
    wbig8 = nc.dram_tensor("wbig8", [NT, 128, K2T, 2, 128], F8, kind="ExternalInput").ap()
    bias = nc.dram_tensor("bias", [128, NT], F32, kind="ExternalInput").ap()
    # c_left/c_right slices packed per h-subtile j: one DMA loads both.
    ccT = nc.dram_tensor("ccT", [JT, 128, 2 * BS], F32, kind="ExternalInput").ap()
    # c (cols 0:BS) and h (cols BS:2BS) packed per h-subtile.
    hcT = nc.dram_tensor("hcT", [JT, 128, 2 * BS], F32, kind="ExternalOutput").ap()

    with tile.TileContext(nc) as tc:
        with (
            tc.tile_pool(name="const", bufs=1) as const_pool,
            tc.tile_pool(name="comb", bufs=1) as comb_pool,
            tc.tile_pool(name="w", bufs=4) as w_pool,
            tc.tile_pool(name="gates", bufs=2) as gate_pool,
            tc.tile_pool(name="cc", bufs=2) as cc_pool,
            tc.tile_pool(name="ew", bufs=1) as ew_pool,
            tc.tile_pool(name="psum", bufs=8, space="PSUM") as psum_pool,
        ):
            bias_sb = const_pool.tile([128, NT], F32, tag="bias")
            nc.scalar.dma_start(bias_sb[:], bias[:])

            # Prefetch the first gates' weights ahead of the bulk comb load
            # so TensorE can start as soon as the first k-tiles land.
            wt_pre = {}
            for g in (4, 0):
                wt = w_pool.tile([128, K2T, 2, 128], F8, tag="wt", name=f"wtpre{g}")
                nc.sync.dma_start(wt[:], wbig8[g * JT])
                wt_pre[g] = wt

            # Persistent comb tiles, one per double-k-tile so matmul deps are
            # per-k2. Split the loads across both DMA queues.
            comb_sb = []
            for k2 in range(K2T):
                ct = comb_pool.tile([128, 2, BS], F8, tag=f"comb{k2}")
                comb_sb.append(ct)
            for k2 in range(K2T):
                eng = nc.scalar if k2 % 2 == 0 else nc.sync
                eng.dma_start(comb_sb[k2][:], comb8[k2])

            def mm_gate(wt, gt, g, n):
                ps = [
                    psum_pool.tile([128, 512], F32, tag="ps", name=f"ps{n}_{bb}")
                    for bb in range(BBT)
                ]
                for k2 in range(K2T):
                    ws = wt[:, k2, :, :]
                    for bb in range(BBT):
                        nc.tensor.matmul(
                            ps[bb][:],
                            ws,
                            comb_sb[k2][:, :, bb * 512:(bb + 1) * 512],
                            start=(k2 == 0),
                            stop=(k2 == K2T - 1),
                            perf_mode=DR,
                        )
                for bb in range(BBT):
                    nc.scalar.activation(
                        gt[:, bb * 512:(bb + 1) * 512],
                        ps[bb][:],
                        AF.Tanh if g == 4 else AF.Sigmoid,
                        bias=bias_sb[:, n:n + 1],
                        scale=SW,
                    )

            # Gate order u,i,fl,fr,o: the c-accumulation on VectorE then
            # overlaps the remaining gates' matmuls. c is stored as soon as
            # it is final (after fr), and the o*tanh(c) product + h store
            # are chunked so only one 512-col chunk trails the last matmul.
            def ew_after(g, gates, c_t, h_t, tmp, cl_t, cr_t, hc_t, j):
                if g == 0:
                    nc.vector.tensor_mul(c_t, gates[0][:], gates[4][:])
                elif g == 1:
                    nc.vector.tensor_mul(tmp[:], gates[1][:], cl_t)
                    nc.vector.tensor_add(c_t, c_t, tmp[:])
                elif g == 2:
                    nc.vector.tensor_mul(tmp[:], gates[2][:], cr_t)
                    nc.vector.tensor_add(c_t, c_t, tmp[:])
                    nc.sync.dma_start(hcT[j][:, 0:BS], c_t)
                elif g == 3:
                    # tanh(c) deferred here so ScalarE runs all sigmoids
                    # back-to-back (one activation-table swap per j, and
                    # the Tanh table carries over into the next j's u gate).
                    for q in range(BBT):
                        qs = slice(q * 512, (q + 1) * 512)
                        nc.scalar.activation(h_t[:, qs], c_t[:, qs], AF.Tanh)
                        nc.vector.tensor_mul(h_t[:, qs], gates[3][:, qs],
                                             h_t[:, qs])
                        nc.sync.dma_start(hcT[j][:, BS + q * 512:BS + (q + 1) * 512],
                                          h_t[:, qs])

            first = True
            for j in [jj for _ in range(repeat) for jj in range(JT)]:
                cc_t = cc_pool.tile([128, 2 * BS], F32, tag="cc")
                nc.scalar.dma_start(cc_t[:], ccT[j])
                cl_t = cc_t[:, 0:BS]
                cr_t = cc_t[:, BS:2 * BS]

                hc_t = ew_pool.tile([128, 2 * BS], F32, tag="hc")
                tmp = ew_pool.tile([128, BS], F32, tag="tmp")
                c_t = hc_t[:, 0:BS]
                h_t = hc_t[:, BS:2 * BS]
                gates = {}

                for g in (4, 0, 1, 2, 3):
                    n = g * JT + j
                    if first and g in wt_pre:
                        wt = wt_pre[g]
                    else:
                        wt = w_pool.tile([128, K2T, 2, 128], F8, tag="wt",
                                         name=f"wt{n}")
                        nc.sync.dma_start(wt[:], wbig8[n])
                    gt = gate_pool.tile([128, BS], F32, tag=f"g{g}",
                                        name=f"g{n}")
                    gates[g] = gt
                    mm_gate(wt, gt, g, n)
                    ew_after(g, gates, c_t, h_t, tmp, cl_t, cr_t, hc_t, j)
                first = False

    nc.compile()
    _NC[repeat] = nc
    return nc


def make_in_maps(x, h_left, c_left, h_right, c_right, W, b):
    x, h_left, c_left, h_right, c_right, W, b = (
        np.asarray(a, dtype=np.float32)
        for a in (x, h_left, c_left, h_right, c_right, W, b)
    )
    comb = np.concatenate([x, h_left, h_right], axis=1)
    comb8 = comb.astype(E4NP)

    # Per TP half t: gate-tile n = g*JT + j covers W rows
    # g*HID + t*HS + j*128 + m. wbig8[n, p, k, m] = Wq[n, m, k*128 + p].
    Wq = (W * (1.0 / SW)).astype(E4NP)
    Wq = Wq.reshape(5, TP, JT * 128, COMB)
    b5 = b.reshape(5, TP, JT, 128)

    in_maps = []
    for i in range(N_CORES):
        dp, t = divmod(i, TP)
        bsl = slice(dp * BS, (dp + 1) * BS)
        # comb8c[k2, p, two, b] = comb[bsl][b, (2*k2+two)*128 + p]
        comb8c = np.ascontiguousarray(
            comb8[bsl].T.reshape(K2T, 2, 128, BS).transpose(0, 2, 1, 3)
        )
        # wcore[n, p, k2, i, m] = Wq[n-th tile row m, (2*k2+i)*128 + p]
        wcore = np.ascontiguousarray(
            Wq[:, t].reshape(NT, 128, K2T, 2, 128).transpose(0, 4, 2, 3, 1)
        )
        bias_arr = np.ascontiguousarray(b5[:, t].reshape(NT, 128).T)
        hsl = slice(t * HS, (t + 1) * HS)
        clT = c_left[bsl, hsl].T.reshape(JT, 128, BS)
        crT = c_right[bsl, hsl].T.reshape(JT, 128, BS)
        in_maps.append({
            "comb8": comb8c,
            "wbig8": wcore,
            "bias": bias_arr,
            "ccT": np.ascontiguousarray(np.concatenate([clT, crT], axis=2)),
        })
    return in_maps


def kernel(x, h_left, c_left, h_right, c_right, W, b):
    nc = _build()
    in_maps = make_in_maps(x, h_left, c_left, h_right, c_right, W, b)
    res = run_bass_kernel_spmd(nc, in_maps, list(range(N_CORES)))
    h = np.empty((B, HID), np.float32)
    c = np.empty((B, HID), np.float32)
    for i in range(N_CORES):
        dp, t = divmod(i, TP)
        bsl = slice(dp * BS, (dp + 1) * BS)
        hsl = slice(t * HS, (t + 1) * HS)
        hc = res.results[i]["hcT"]  # (JT, 128, 2*BS)
        c[bsl, hsl] = hc[:, :, :BS].transpose(2, 0, 1).reshape(BS, HS)
        h[bsl, hsl] = hc[:, :, BS:].transpose(2, 0, 1).reshape(BS, HS)
    return h, c


# revision 19
# speedup vs baseline: 1.0347x; 1.0347x over previous
"""BinaryTreeLSTMCell fused kernel for 8 TRN2 NeuronCores.

Strategy: 2D sharding — 4-way data-parallel over the batch x 2-way
tensor-parallel over the hidden (output) dim, so each core computes a
(2048 batch, 512 h) block of the output with no cross-core traffic.
Per core, gates^T = W_half @ [x|h_left|h_right]^T runs in fp8-E4M3
with perf_mode=DoubleRow (2 weights per PE cell, K=256 per matmul,
~1.8x bf16 throughput). Weights are pre-scaled by 2^10 on host so
their U(-1/sqrt(3072), ..) range clears e4m3's min-normal (0.0156);
the 2^-10 dequant rides the activation's scale operand together with
the bias add. Loop order k2-outer / moving-block-inner amortizes each
256-column LDWEIGHTS over 4 moving blocks. Gate nonlinearities fused
on ScalarE out of PSUM, LSTM cell elementwise on VectorE in fp32.
"""

import numpy as np
import ml_dtypes

import concourse.bacc as bacc
import concourse.mybir as mybir
import concourse.tile as tile
from concourse.bass_utils import run_bass_kernel_spmd

F32 = mybir.dt.float32
F8 = mybir.dt.float8e4
E4NP = ml_dtypes.float8_e4m3
AF = mybir.ActivationFunctionType
DR = mybir.MatmulPerfMode.DoubleRow

N_CORES = 8
B = 8192
IN_SIZE = 1024
HID = 1024
COMB = IN_SIZE + 2 * HID          # 3072 contraction dim
DP = 4                            # batch shards
TP = 2                            # hidden shards
BS = B // DP                      # 2048 batch rows per core
HS = HID // TP                    # 512 h-cols per core
KT = COMB // 128                  # 24 k-tiles
K2T = KT // 2                     # 12 double-k-tiles (256 each)
NT = 5 * HS // 128                # 20 gate tiles per core
JT = HS // 128                    # 4 h-subtiles per core
BBT = BS // 512                   # 4 moving blocks of 512
SW = 2.0 ** -10                   # weight dequant scale

_NC = {}


def _build(repeat=1):
    if repeat in _NC:
        return _NC[repeat]

    nc = bacc.Bacc("TRN2", target_bir_lowering=False, debug=False)

    # comb8[k2] = [128 kpart, 2 kplanes, BS batch] fp8 per double-k-tile.
    comb8 = nc.dram_tensor("comb8", [K2T, 128, 2, BS], F8, kind="ExternalInput").ap()
    # wbig8[n] = [128 kpart, 12 k2, 2 kplane, 128 m] fp8 per gate tile.
    wbig8 = nc.dram_tensor("wbig8", [NT, 128, K2T, 2, 128], F8, kind="ExternalInput").ap()
    bias = nc.dram_tensor("bias", [128, NT], F32, kind="ExternalInput").ap()
    # c_left/c_right slices packed per h-subtile j: one DMA loads both.
    ccT = nc.dram_tensor("ccT", [JT, 128, 2 * BS], F32, kind="ExternalInput").ap()
    # c (cols 0:BS) and h (cols BS:2BS) packed per h-subtile.
    hcT = nc.dram_tensor("hcT", [JT, 128, 2 * BS], F32, kind="ExternalOutput").ap()

    with tile.TileContext(nc) as tc:
        with (
            tc.tile_pool(name="const", bufs=1) as const_pool,
            tc.tile_pool(name="comb", bufs=1) as comb_pool,
            tc.tile_pool(name="w", bufs=3) as w_pool,
            tc.tile_pool(name="gates", bufs=2) as gate_pool,
            tc.tile_pool(name="cc", bufs=2) as cc_pool,
            tc.tile_pool(name="ew", bufs=1) as ew_pool,
            tc.tile_pool(name="psum", bufs=8, space="PSUM") as psum_pool,
        ):
            bias_sb = const_pool.tile([128, NT], F32, tag="bias")
            nc.scalar.dma_start(bias_sb[:], bias[:])

            # Prefetch the first gates' weights ahead of the bulk comb load
            # so TensorE can start as soon as the first k-tiles land.
            wt_pre = {}
            for g in (4, 0):
                wt = w_pool.tile([128, K2T, 2, 128], F8, tag="wt", name=f"wtpre{g}")
                nc.sync.dma_start(wt[:], wbig8[g * JT])
                wt_pre[g] = wt

            # Persistent comb tiles, one per double-k-tile so matmul deps are
            # per-k2. Split the loads across both DMA queues.
            comb_sb = []
            for k2 in range(K2T):
                ct = comb_pool.tile([128, 2, BS], F8, tag=f"comb{k2}")
                comb_sb.append(ct)
            for k2 in range(K2T):
                eng = nc.scalar if k2 % 2 == 0 else nc.sync
                eng.dma_start(comb_sb[k2][:], comb8[k2])

            def mm_gate(wt, gt, g, n):
                ps = [
                    psum_pool.tile([128, 512], F32, tag="ps", name=f"ps{n}_{bb}")
                    for bb in range(BBT)
                ]
                for k2 in range(K2T):
                    ws = wt[:, k2, :, :]
                    for bb in range(BBT):
                        nc.tensor.matmul(
                            ps[bb][:],
                            ws,
                            comb_sb[k2][:, :, bb * 512:(bb + 1) * 512],
                            start=(k2 == 0),
                            stop=(k2 == K2T - 1),
                            perf_mode=DR,
                        )
                for bb in range(BBT):
                    nc.scalar.activation(
                        gt[:, bb * 512:(bb + 1) * 512],
                        ps[bb][:],
                        AF.Tanh if g == 4 else AF.Sigmoid,
                        bias=bias_sb[:, n:n + 1],
                        scale=SW,
                    )

            # Gate order u,i,fl,fr,o: the c-accumulation on VectorE then
            # overlaps the remaining gates' matmuls. c is stored as soon as
            # it is final (after fr), and the o*tanh(c) product + h store
            # are chunked so only one 512-col chunk trails the last matmul.
            def ew_after(g, gates, c_t, h_t, tmp, cl_t, cr_t, hc_t, j):
                if g == 0:
                    nc.vector.tensor_mul(c_t, gates[0][:], gates[4][:])
                elif g == 1:
                    nc.vector.tensor_mul(tmp[:], gates[1][:], cl_t)
                    nc.vector.tensor_add(c_t, c_t, tmp[:])
                elif g == 2:
                    nc.vector.tensor_mul(tmp[:], gates[2][:], cr_t)
                    nc.vector.tensor_add(c_t, c_t, tmp[:])
                    nc.scalar.activation(h_t, c_t, AF.Tanh)
                    nc.sync.dma_start(hcT[j][:, 0:BS], c_t)
                elif g == 3:
                    for q in range(BBT):
                        qs = slice(q * 512, (q + 1) * 512)
                        nc.vector.tensor_mul(h_t[:, qs], gates[3][:, qs],
                                             h_t[:, qs])
                        nc.sync.dma_start(hcT[j][:, BS + q * 512:BS + (q + 1) * 512],
                                          h_t[:, qs])

            first = True
            for j in [jj for _ in range(repeat) for jj in range(JT)]:
                cc_t = cc_pool.tile([128, 2 * BS], F32, tag="cc")
                nc.scalar.dma_start(cc_t[:], ccT[j])
                cl_t = cc_t[:, 0:BS]
                cr_t = cc_t[:, BS:2 * BS]

                hc_t = ew_pool.tile([128, 2 * BS], F32, tag="hc")
                tmp = ew_pool.tile([128, BS], F32, tag="tmp")
                c_t = hc_t[:, 0:BS]
                h_t = hc_t[:, BS:2 * BS]
                gates = {}

                for g in (4, 0, 1, 2, 3):
                    n = g * JT + j
                    if first and g in wt_pre:
                        wt = wt_pre[g]
                    else:
                        wt = w_pool.tile([128, K2T, 2, 128], F8, tag="wt",
                                         name=f"wt{n}")
                        nc.sync.dma_start(wt[:], wbig8[n])
                    gt = gate_pool.tile([128, BS], F32, tag=f"g{g}",
                                        name=f"g{n}")
                    gates[g] = gt
                    mm_gate(wt, gt, g, n)
                    ew_after(g, gates, c_t, h_t, tmp, cl_t, cr_t, hc_t, j)
                first = False

    nc.compile()
    _NC[repeat] = nc
    return nc


def make_in_maps(x, h_left, c_left, h_right, c_right, W, b):
    x, h_left, c_left, h_right, c_right, W, b = (
        np.asarray(a, dtype=np.float32)
        for a in (x, h_left, c_left, h_right, c_right, W, b)
    )
    comb = np.concatenate([x, h_left, h_right], axis=1)
    comb8 = comb.astype(E4NP)

    # Per TP half t: gate-tile n = g*JT + j covers W rows
    # g*HID + t*HS + j*128 + m. wbig8[n, p, k, m] = Wq[n, m, k*128 + p].
    Wq = (W * (1.0 / SW)).astype(E4NP)
    Wq = Wq.reshape(5, TP, JT * 128, COMB)
    b5 = b.reshape(5, TP, JT, 128)

    in_maps = []
    for i in range(N_CORES):
        dp, t = divmod(i, TP)
        bsl = slice(dp * BS, (dp + 1) * BS)
        # comb8c[k2, p, two, b] = comb[bsl][b, (2*k2+two)*128 + p]
        comb8c = np.ascontiguousarray(
            comb8[bsl].T.reshape(K2T, 2, 128, BS).transpose(0, 2, 1, 3)
        )
        # wcore[n, p, k2, i, m] = Wq[n-th tile row m, (2*k2+i)*128 + p]
        wcore = np.ascontiguousarray(
            Wq[:, t].reshape(NT, 128, K2T, 2, 128).transpose(0, 4, 2, 3, 1)
        )
        bias_arr = np.ascontiguousarray(b5[:, t].reshape(NT, 128).T)
        hsl = slice(t * HS, (t + 1) * HS)
        clT = c_left[bsl, hsl].T.reshape(JT, 128, BS)
        crT = c_right[bsl, hsl].T.reshape(JT, 128, BS)
        in_maps.append({
            "comb8": comb8c,
            "wbig8": wcore,
            "bias": bias_arr,
            "ccT": np.ascontiguousarray(np.concatenate([clT, crT], axis=2)),
        })
    return in_maps


def kernel(x, h_left, c_left, h_right, c_right, W, b):
    nc = _build()
    in_maps = make_in_maps(x, h_left, c_left, h_right, c_right, W, b)
    res = run_bass_kernel_spmd(nc, in_maps, list(range(N_CORES)))
    h = np.empty((B, HID), np.float32)
    c = np.empty((B, HID), np.float32)
    for i in range(N_CORES):
        dp, t = divmod(i, TP)
        bsl = slice(dp * BS, (dp + 1) * BS)
        hsl = slice(t * HS, (t + 1) * HS)
        hc = res.results[i]["hcT"]  # (JT, 128, 2*BS)
        c[bsl, hsl] = hc[:, :, :BS].transpose(2, 0, 1).reshape(BS, HS)
        h[bsl, hsl] = hc[:, :, BS:].transpose(2, 0, 1).reshape(BS, HS)
    return h, c


# revision 21
# speedup vs baseline: 1.0373x; 1.0025x over previous
"""BinaryTreeLSTMCell fused kernel for 8 TRN2 NeuronCores.

Strategy: 2D sharding — 4-way data-parallel over the batch x 2-way
tensor-parallel over the hidden (output) dim, so each core computes a
(2048 batch, 512 h) block of the output with no cross-core traffic.
Per core, gates^T = W_half @ [x|h_left|h_right]^T runs in fp8-E4M3
with perf_mode=DoubleRow (2 weights per PE cell, K=256 per matmul,
~1.8x bf16 throughput). Weights are pre-scaled by 2^10 on host so
their U(-1/sqrt(3072), ..) range clears e4m3's min-normal (0.0156);
the 2^-10 dequant rides the activation's scale operand together with
the bias add. Loop order k2-outer / moving-block-inner amortizes each
256-column LDWEIGHTS over 4 moving blocks. Gate nonlinearities fused
on ScalarE out of PSUM, LSTM cell elementwise on VectorE in fp32.
"""

import numpy as np
import ml_dtypes

import concourse.bacc as bacc
import concourse.mybir as mybir
import concourse.tile as tile
from concourse.bass_utils import run_bass_kernel_spmd

F32 = mybir.dt.float32
F8 = mybir.dt.float8e4
E4NP = ml_dtypes.float8_e4m3
AF = mybir.ActivationFunctionType
DR = mybir.MatmulPerfMode.DoubleRow

N_CORES = 8
B = 8192
IN_SIZE = 1024
HID = 1024
COMB = IN_SIZE + 2 * HID          # 3072 contraction dim
DP = 4                            # batch shards
TP = 2                            # hidden shards
BS = B // DP                      # 2048 batch rows per core
HS = HID // TP                    # 512 h-cols per core
KT = COMB // 128                  # 24 k-tiles
K2T = KT // 2                     # 12 double-k-tiles (256 each)
NT = 5 * HS // 128                # 20 gate tiles per core
JT = HS // 128                    # 4 h-subtiles per core
BBT = BS // 512                   # 4 moving blocks of 512
SW = 2.0 ** -10                   # weight dequant scale

_NC = {}


def _build(repeat=1):
    if repeat in _NC:
        return _NC[repeat]

    nc = bacc.Bacc("TRN2", target_bir_lowering=False, debug=False)

    # comb8[k2] = [128 kpart, 2 kplanes, BS batch] fp8 per double-k-tile.
    comb8 = nc.dram_tensor("comb8", [K2T, 128, 2, BS], F8, kind="ExternalInput").ap()
    # wbig8[n] = [128 kpart, 12 k2, 2 kplane, 128 m] fp8 per gate tile.
    wbig8 = nc.dram_tensor("wbig8", [NT, 128, K2T, 2, 128], F8, kind="ExternalInput").ap()
    bias = nc.dram_tensor("bias", [128, NT], F32, kind="ExternalInput").ap()
    # c_left/c_right slices packed per h-subtile j: one DMA loads both.
    ccT = nc.dram_tensor("ccT", [JT, 128, 2 * BS], F32, kind="ExternalInput").ap()
    # c (cols 0:BS) and h (cols BS:2BS) packed per h-subtile.
    hcT = nc.dram_tensor("hcT", [JT, 128, 2 * BS], F32, kind="ExternalOutput").ap()

    with tile.TileContext(nc) as tc:
        with (
            tc.tile_pool(name="const", bufs=1) as const_pool,
            tc.tile_pool(name="comb", bufs=1) as comb_pool,
            tc.tile_pool(name="w", bufs=3) as w_pool,
            tc.tile_pool(name="gates", bufs=2) as gate_pool,
            tc.tile_pool(name="cc", bufs=2) as cc_pool,
            tc.tile_pool(name="ew", bufs=1) as ew_pool,
            tc.tile_pool(name="psum", bufs=8, space="PSUM") as psum_pool,
        ):
            bias_sb = const_pool.tile([128, NT], F32, tag="bias")
            nc.scalar.dma_start(bias_sb[:], bias[:])

            # Prefetch the first gates' weights ahead of the bulk comb load
            # so TensorE can start as soon as the first k-tiles land.
            wt_pre = {}
            for g in (4, 0):
                wt = w_pool.tile([128, K2T, 2, 128], F8, tag="wt", name=f"wtpre{g}")
                nc.sync.dma_start(wt[:], wbig8[g * JT])
                wt_pre[g] = wt

            # Persistent comb tiles, one per double-k-tile so matmul deps are
            # per-k2. Split the loads across both DMA queues.
            comb_sb = []
            for k2 in range(K2T):
                ct = comb_pool.tile([128, 2, BS], F8, tag=f"comb{k2}")
                comb_sb.append(ct)
            for k2 in range(K2T):
                eng = nc.scalar if k2 % 2 == 0 else nc.sync
                eng.dma_start(comb_sb[k2][:], comb8[k2])

            def mm_gate(wt, gt, g, n):
                ps = [
                    psum_pool.tile([128, 512], F32, tag="ps", name=f"ps{n}_{bb}")
                    for bb in range(BBT)
                ]
                for k2 in range(K2T):
                    ws = wt[:, k2, :, :]
                    for bb in range(BBT):
                        nc.tensor.matmul(
                            ps[bb][:],
                            ws,
                            comb_sb[k2][:, :, bb * 512:(bb + 1) * 512],
                            start=(k2 == 0),
                            stop=(k2 == K2T - 1),
                            perf_mode=DR,
                        )
                for bb in range(BBT):
                    nc.scalar.activation(
                        gt[:, bb * 512:(bb + 1) * 512],
                        ps[bb][:],
                        AF.Tanh if g == 4 else AF.Sigmoid,
                        bias=bias_sb[:, n:n + 1],
                        scale=SW,
                    )

            # Gate order u,i,fl,fr,o: the c-accumulation on VectorE then
            # overlaps the remaining gates' matmuls. c is stored as soon as
            # it is final (after fr), and the o*tanh(c) product + h store
            # are chunked so only one 512-col chunk trails the last matmul.
            def ew_after(g, gates, c_t, h_t, tmp, cl_t, cr_t, hc_t, j):
                if g == 0:
                    nc.vector.tensor_mul(c_t, gates[0][:], gates[4][:])
                elif g == 1:
                    nc.vector.tensor_mul(tmp[:], gates[1][:], cl_t)
                    nc.vector.tensor_add(c_t, c_t, tmp[:])
                elif g == 2:
                    nc.vector.tensor_mul(tmp[:], gates[2][:], cr_t)
                    nc.vector.tensor_add(c_t, c_t, tmp[:])
                    nc.scalar.activation(h_t, c_t, AF.Tanh)
                    nc.scalar.dma_start(hcT[j][:, 0:BS], c_t)
                elif g == 3:
                    for q in range(BBT):
                        qs = slice(q * 512, (q + 1) * 512)
                        nc.vector.tensor_mul(h_t[:, qs], gates[3][:, qs],
                                             h_t[:, qs])
                        nc.scalar.dma_start(hcT[j][:, BS + q * 512:BS + (q + 1) * 512],
                                            h_t[:, qs])

            first = True
            for j in [jj for _ in range(repeat) for jj in range(JT)]:
                cc_t = cc_pool.tile([128, 2 * BS], F32, tag="cc")
                nc.sync.dma_start(cc_t[:], ccT[j])
                cl_t = cc_t[:, 0:BS]
                cr_t = cc_t[:, BS:2 * BS]

                hc_t = ew_pool.tile([128, 2 * BS], F32, tag="hc")
                tmp = ew_pool.tile([128, BS], F32, tag="tmp")
                c_t = hc_t[:, 0:BS]
                h_t = hc_t[:, BS:2 * BS]
                gates = {}

                for g in (4, 0, 1, 2, 3):
                    n = g * JT + j
                    if first and g in wt_pre:
                        wt = wt_pre[g]
                    else:
                        wt = w_pool.tile([128, K2T, 2, 128], F8, tag="wt",
                                         name=f"wt{n}")
                        nc.sync.dma_start(wt[:], wbig8[n])
                    gt = gate_pool.tile([128, BS], F32, tag=f"g{g}",
                                        name=f"g{n}")
                    gates[g] = gt
                    mm_gate(wt, gt, g, n)
                    ew_after(g, gates, c_t, h_t, tmp, cl_t, cr_t, hc_t, j)
                first = False

    nc.compile()
    _NC[repeat] = nc
    return nc


def make_in_maps(x, h_left, c_left, h_right, c_right, W, b):
    x, h_left, c_left, h_right, c_right, W, b = (
        np.asarray(a, dtype=np.float32)
        for a in (x, h_left, c_left, h_right, c_right, W, b)
    )
    comb = np.concatenate([x, h_left, h_right], axis=1)
    comb8 = comb.astype(E4NP)

    # Per TP half t: gate-tile n = g*JT + j covers W rows
    # g*HID + t*HS + j*128 + m. wbig8[n, p, k, m] = Wq[n, m, k*128 + p].
    Wq = (W * (1.0 / SW)).astype(E4NP)
    Wq = Wq.reshape(5, TP, JT * 128, COMB)
    b5 = b.reshape(5, TP, JT, 128)

    in_maps = []
    for i in range(N_CORES):
        dp, t = divmod(i, TP)
        bsl = slice(dp * BS, (dp + 1) * BS)
        # comb8c[k2, p, two, b] = comb[bsl][b, (2*k2+two)*128 + p]
        comb8c = np.ascontiguousarray(
            comb8[bsl].T.reshape(K2T, 2, 128, BS).transpose(0, 2, 1, 3)
        )
        # wcore[n, p, k2, i, m] = Wq[n-th tile row m, (2*k2+i)*128 + p]
        wcore = np.ascontiguousarray(
            Wq[:, t].reshape(NT, 128, K2T, 2, 128).transpose(0, 4, 2, 3, 1)
        )
        bias_arr = np.ascontiguousarray(b5[:, t].reshape(NT, 128).T)
        hsl = slice(t * HS, (t + 1) * HS)
        clT = c_left[bsl, hsl].T.reshape(JT, 128, BS)
        crT = c_right[bsl, hsl].T.reshape(JT, 128, BS)
        in_maps.append({
            "comb8": comb8c,
            "wbig8": wcore,
            "bias": bias_arr,
            "ccT": np.ascontiguousarray(np.concatenate([clT, crT], axis=2)),
        })
    return in_maps


def kernel(x, h_left, c_left, h_right, c_right, W, b):
    nc = _build()
    in_maps = make_in_maps(x, h_left, c_left, h_right, c_right, W, b)
    res = run_bass_kernel_spmd(nc, in_maps, list(range(N_CORES)))
    h = np.empty((B, HID), np.float32)
    c = np.empty((B, HID), np.float32)
    for i in range(N_CORES):
        dp, t = divmod(i, TP)
        bsl = slice(dp * BS, (dp + 1) * BS)
        hsl = slice(t * HS, (t + 1) * HS)
        hc = res.results[i]["hcT"]  # (JT, 128, 2*BS)
        c[bsl, hsl] = hc[:, :, :BS].transpose(2, 0, 1).reshape(BS, HS)
        h[bsl, hsl] = hc[:, :, BS:].transpose(2, 0, 1).reshape(BS, HS)
    return h, c


# revision 23
# speedup vs baseline: 1.2754x; 1.2295x over previous
"""BinaryTreeLSTMCell fused kernel for 8 TRN2 NeuronCores.

Strategy: 2D sharding — 4-way data-parallel over the batch x 2-way
tensor-parallel over the hidden (output) dim, so each core computes a
(2048 batch, 512 h) block of the output with no cross-core traffic.
Per core, gates^T = W_half @ [x|h_left|h_right]^T runs in fp8-E4M3
with perf_mode=DoubleRow (2 weights per PE cell, K=256 per matmul,
~1.8x bf16 throughput). Weights are pre-scaled by 2^10 on host so
their U(-1/sqrt(3072), ..) range clears e4m3's min-normal (0.0156);
the 2^-10 dequant rides the activation's scale operand together with
the bias add. Loop order k2-outer / moving-block-inner amortizes each
256-column LDWEIGHTS over 4 moving blocks. Gate nonlinearities fused
on ScalarE out of PSUM, LSTM cell elementwise on VectorE in fp32.
"""

import numpy as np
import ml_dtypes

import concourse.bacc as bacc
import concourse.mybir as mybir
import concourse.tile as tile
from concourse.bass_utils import run_bass_kernel_spmd

F32 = mybir.dt.float32
F8 = mybir.dt.float8e4
E4NP = ml_dtypes.float8_e4m3
AF = mybir.ActivationFunctionType
DR = mybir.MatmulPerfMode.DoubleRow

N_CORES = 8
B = 8192
IN_SIZE = 1024
HID = 1024
COMB = IN_SIZE + 2 * HID          # 3072 contraction dim
DP = 4                            # batch shards
TP = 2                            # hidden shards
BS = B // DP                      # 2048 batch rows per core
HS = HID // TP                    # 512 h-cols per core
KT = COMB // 128                  # 24 k-tiles
K2T = KT // 2                     # 12 double-k-tiles (256 each)
NT = 5 * HS // 128                # 20 gate tiles per core
JT = HS // 128                    # 4 h-subtiles per core
BBT = BS // 512                   # 4 moving blocks of 512
SW = 2.0 ** -10                   # weight dequant scale

_NC = {}


def _build(repeat=1):
    if repeat in _NC:
        return _NC[repeat]

    nc = bacc.Bacc("TRN2", target_bir_lowering=False, debug=False)

    # comb8[k2] = [128 kpart, 2 kplanes, BS batch] fp8 per double-k-tile.
    comb8 = nc.dram_tensor("comb8", [K2T, 128, 2, BS], F8, kind="ExternalInput").ap()
    # wbig8[n] = [128 kpart, 12 k2, 2 kplane, 128 m] fp8 per gate tile.
    wbig8 = nc.dram_tensor("wbig8", [NT, 128, K2T, 2, 128], F8, kind="ExternalInput").ap()
    bias = nc.dram_tensor("bias", [128, NT], F32, kind="ExternalInput").ap()
    # c_left/c_right slices packed per h-subtile j: one DMA loads both.
    ccT = nc.dram_tensor("ccT", [JT, 128, 2 * BS], F32, kind="ExternalInput").ap()
    # c (cols 0:BS) and h (cols BS:2BS) packed per h-subtile.
    hcT = nc.dram_tensor("hcT", [JT, 128, 2 * BS], F32, kind="ExternalOutput").ap()

    with tile.TileContext(nc) as tc:
        with (
            tc.tile_pool(name="const", bufs=1) as const_pool,
            tc.tile_pool(name="comb", bufs=1) as comb_pool,
            tc.tile_pool(name="w", bufs=3) as w_pool,
            tc.tile_pool(name="gates", bufs=2) as gate_pool,
            tc.tile_pool(name="cc", bufs=2) as cc_pool,
            tc.tile_pool(name="ew", bufs=1) as ew_pool,
            tc.tile_pool(name="psum", bufs=8, space="PSUM") as psum_pool,
        ):
            bias_sb = const_pool.tile([128, NT], F32, tag="bias")
            nc.scalar.dma_start(bias_sb[:], bias[:])

            # Prefetch the first gates' weights ahead of the bulk comb load
            # so TensorE can start as soon as the first k-tiles land.
            wt_pre = {}
            for g in (4, 0):
                wt = w_pool.tile([128, K2T, 2, 128], F8, tag="wt", name=f"wtpre{g}")
                nc.sync.dma_start(wt[:], wbig8[g * JT])
                wt_pre[g] = wt

            # Persistent comb tiles, one per double-k-tile so matmul deps are
            # per-k2. Split the loads across both DMA queues.
            comb_sb = []
            for k2 in range(K2T):
                ct = comb_pool.tile([128, 2, BS], F8, tag=f"comb{k2}")
                comb_sb.append(ct)
            for k2 in range(K2T):
                eng = nc.scalar if k2 % 2 == 0 else nc.sync
                eng.dma_start(comb_sb[k2][:], comb8[k2])

            # bb-outer: consecutive matmuls accumulate into the same PSUM
            # bank (no bank alternation), and each bank's activation drains
            # while the next bank's chain runs. LDWEIGHTS is pipelined by
            # the PE regardless of reload count.
            def mm_gate(wt, gt, g, n):
                for bb in range(BBT):
                    ps = psum_pool.tile([128, 512], F32, tag="ps",
                                        name=f"ps{n}_{bb}")
                    for k2 in range(K2T):
                        nc.tensor.matmul(
                            ps[:],
                            wt[:, k2, :, :],
                            comb_sb[k2][:, :, bb * 512:(bb + 1) * 512],
                            start=(k2 == 0),
                            stop=(k2 == K2T - 1),
                            perf_mode=DR,
                        )
                    nc.scalar.activation(
                        gt[:, bb * 512:(bb + 1) * 512],
                        ps[:],
                        AF.Tanh if g == 4 else AF.Sigmoid,
                        bias=bias_sb[:, n:n + 1],
                        scale=SW,
                    )

            # Gate order u,i,fl,fr,o: the c-accumulation on VectorE then
            # overlaps the remaining gates' matmuls. c is stored as soon as
            # it is final (after fr), and the o*tanh(c) product + h store
            # are chunked so only one 512-col chunk trails the last matmul.
            def ew_after(g, gates, c_t, h_t, tmp, cl_t, cr_t, hc_t, j):
                if g == 0:
                    nc.vector.tensor_mul(c_t, gates[0][:], gates[4][:])
                elif g == 1:
                    nc.vector.tensor_mul(tmp[:], gates[1][:], cl_t)
                    nc.vector.tensor_add(c_t, c_t, tmp[:])
                elif g == 2:
                    nc.vector.tensor_mul(tmp[:], gates[2][:], cr_t)
                    nc.vector.tensor_add(c_t, c_t, tmp[:])
                    nc.scalar.activation(h_t, c_t, AF.Tanh)
                    nc.scalar.dma_start(hcT[j][:, 0:BS], c_t)
                elif g == 3:
                    for q in range(BBT):
                        qs = slice(q * 512, (q + 1) * 512)
                        nc.vector.tensor_mul(h_t[:, qs], gates[3][:, qs],
                                             h_t[:, qs])
                        nc.scalar.dma_start(hcT[j][:, BS + q * 512:BS + (q + 1) * 512],
                                            h_t[:, qs])

            first = True
            for j in [jj for _ in range(repeat) for jj in range(JT)]:
                cc_t = cc_pool.tile([128, 2 * BS], F32, tag="cc")
                nc.sync.dma_start(cc_t[:], ccT[j])
                cl_t = cc_t[:, 0:BS]
                cr_t = cc_t[:, BS:2 * BS]

                hc_t = ew_pool.tile([128, 2 * BS], F32, tag="hc")
                tmp = ew_pool.tile([128, BS], F32, tag="tmp")
                c_t = hc_t[:, 0:BS]
                h_t = hc_t[:, BS:2 * BS]
                gates = {}

                for g in (4, 0, 1, 2, 3):
                    n = g * JT + j
                    if first and g in wt_pre:
                        wt = wt_pre[g]
                    else:
                        wt = w_pool.tile([128, K2T, 2, 128], F8, tag="wt",
                                         name=f"wt{n}")
                        nc.sync.dma_start(wt[:], wbig8[n])
                    gt = gate_pool.tile([128, BS], F32, tag=f"g{g}",
                                        name=f"g{n}")
                    gates[g] = gt
                    mm_gate(wt, gt, g, n)
                    ew_after(g, gates, c_t, h_t, tmp, cl_t, cr_t, hc_t, j)
                first = False

    nc.compile()
    _NC[repeat] = nc
    return nc


def make_in_maps(x, h_left, c_left, h_right, c_right, W, b):
    x, h_left, c_left, h_right, c_right, W, b = (
        np.asarray(a, dtype=np.float32)
        for a in (x, h_left, c_left, h_right, c_right, W, b)
    )
    comb = np.concatenate([x, h_left, h_right], axis=1)
    comb8 = comb.astype(E4NP)

    # Per TP half t: gate-tile n = g*JT + j covers W rows
    # g*HID + t*HS + j*128 + m. wbig8[n, p, k, m] = Wq[n, m, k*128 + p].
    Wq = (W * (1.0 / SW)).astype(E4NP)
    Wq = Wq.reshape(5, TP, JT * 128, COMB)
    b5 = b.reshape(5, TP, JT, 128)

    in_maps = []
    for i in range(N_CORES):
        dp, t = divmod(i, TP)
        bsl = slice(dp * BS, (dp + 1) * BS)
        # comb8c[k2, p, two, b] = comb[bsl][b, (2*k2+two)*128 + p]
        comb8c = np.ascontiguousarray(
            comb8[bsl].T.reshape(K2T, 2, 128, BS).transpose(0, 2, 1, 3)
        )
        # wcore[n, p, k2, i, m] = Wq[n-th tile row m, (2*k2+i)*128 + p]
        wcore = np.ascontiguousarray(
            Wq[:, t].reshape(NT, 128, K2T, 2, 128).transpose(0, 4, 2, 3, 1)
        )
        bias_arr = np.ascontiguousarray(b5[:, t].reshape(NT, 128).T)
        hsl = slice(t * HS, (t + 1) * HS)
        clT = c_left[bsl, hsl].T.reshape(JT, 128, BS)
        crT = c_right[bsl, hsl].T.reshape(JT, 128, BS)
        in_maps.append({
            "comb8": comb8c,
            "wbig8": wcore,
            "bias": bias_arr,
            "ccT": np.ascontiguousarray(np.concatenate([clT, crT], axis=2)),
        })
    return in_maps


def kernel(x, h_left, c_left, h_right, c_right, W, b):
    nc = _build()
    in_maps = make_in_maps(x, h_left, c_left, h_right, c_right, W, b)
    res = run_bass_kernel_spmd(nc, in_maps, list(range(N_CORES)))
    h = np.empty((B, HID), np.float32)
    c = np.empty((B, HID), np.float32)
    for i in range(N_CORES):
        dp, t = divmod(i, TP)
        bsl = slice(dp * BS, (dp + 1) * BS)
        hsl = slice(t * HS, (t + 1) * HS)
        hc = res.results[i]["hcT"]  # (JT, 128, 2*BS)
        c[bsl, hsl] = hc[:, :, :BS].transpose(2, 0, 1).reshape(BS, HS)
        h[bsl, hsl] = hc[:, :, BS:].transpose(2, 0, 1).reshape(BS, HS)
    return h, c
